# revision 22
# baseline (speedup 1.0000x reference)
"""Trainium2 Bass kernel for nn_DebiasIntraDist (segment_reduce).

Full-input contract: kernel(**inputs) takes the complete (unsharded) inputs
and returns the full scalar loss. The N=65536 samples are sharded across the
8 NeuronCores by (demog, label-half): core 2d+h gets the rows with
demog == d and label-half h (a partition of the N axis). Every core then
owns a disjoint set of 256 (demog, label) groups, so no cross-core
reduction of group accumulators is needed at all.

Design (v5):
  * feats are quantized to fp16 on the host -> HBM traffic halves (the
    DMA floor is ~24 us/core). All on-device arithmetic on the quantized
    data is fp32-accumulated, so the only meaningful error is the fp16
    input rounding itself (~5e-4 relative on the loss).
  * within each core, rows are bucketed by group-chunk (local group id
    <128 vs >=128) so every 128-row tile touches a single 128-wide
    one-hot chunk -> each feature element is streamed through the PE
    exactly twice (sums and squares), not 4x as in the v1 baseline.
  * per-group sumsq uses a second PE matmul over Y = X*X instead of
    per-row fused reductions on DVE/ACT (those run at 1x, ~600-930 ns
    per tile; the elementwise square is 2x-mode on DVE). Y production is
    column-split between the Vector and Scalar engines so both stay
    under the DMA floor.
  * group counts are sharding metadata; the host knows them exactly
    (bincount), so the device only accumulates sums[g,:] and sumsq[g].
  * no collective: each core DMAs out its 256 groups' statistics and the
    host does the final 2048-group scalar reduction (the v1 AllGather
    mesh cost ~35 us of serial tail).

Math per group: sums[g, :] and sumsqvec[g, :] (one-hot matmuls), then
    sum_{i in g} ||x_i - mu_g||^2 = sum_d sumsqvec[g,d] - ||sums[g]||^2 / cnt[g].
"""

import numpy as np

try:
    import concourse.bacc as bacc
except ImportError:  # fresh environment without PYTHONPATH set up
    import sys
    for p in ("/root/.axon_site/_ro/trn_rl_repo", "/opt/trn_rl_repo",
              "/root/.axon_site/_ro/pypackages"):
        if p not in sys.path:
            sys.path.append(p)
    import concourse.bacc as bacc
import concourse.mybir as mybir
import concourse.tile as tile
import concourse.bass_utils as bass_utils

N_CORES = 8
P = 128
D = 512          # feature dim
NL = 256         # labels per core after (demog, label-half) sharding
ND = 4           # demog values
CH = 6           # sample-tiles per feats DMA (768 KiB)
PAD_LAB = 300.0  # pad label; never matches iota 0..127
Y_WV = 170       # columns of Y squared on VectorE (rest: ScalarE)

_cache: dict[tuple, object] = {}


def _build(T0: int, T1: int):
    """Compile the SPMD kernel: T0 tiles of chunk 0 then T1 of chunk 1."""
    T = T0 + T1
    fp32 = mybir.dt.float32
    fp16 = mybir.dt.float16
    Alu = mybir.AluOpType
    Act = mybir.ActivationFunctionType

    nc = bacc.Bacc("TRN2", target_bir_lowering=False, debug=False,
                   enable_asserts=True, num_devices=N_CORES)

    feats_t = nc.dram_tensor("feats_t", [P, T, D], fp16,
                             kind="ExternalInput").ap()
    labels_t = nc.dram_tensor("labels_t", [P, T], fp32,
                              kind="ExternalInput").ap()
    stats = nc.dram_tensor("stats", [P, 4], fp32, kind="ExternalOutput").ap()

    with tile.TileContext(nc) as tc:
        with (
            tc.tile_pool(name="const", bufs=1) as constp,
            tc.tile_pool(name="fx", bufs=12) as fxp,
            tc.tile_pool(name="oh", bufs=16) as ohp,
            tc.tile_pool(name="yy", bufs=3) as yyp,
            tc.tile_pool(name="yya", bufs=3) as yyap,
            tc.tile_pool(name="post", bufs=1) as postp,
            tc.tile_pool(name="ps", bufs=1, space="PSUM") as psp,
        ):
            # labels go out first on the sync ring: the one-hots (and so
            # every matmul) gate on them, and a trigger costs the ring
            # nothing; keeping them OFF the scalar ring matters because
            # the scheduler parks DMA triggers there behind 2us squares
            labs = constp.tile([P, T], fp32, tag="labs")
            nc.sync.dma_start(out=labs[:], in_=labels_t[:])
            # constants
            iota = constp.tile([P, P], fp16, tag="iota")
            nc.gpsimd.iota(iota[:], [[1, P]], channel_multiplier=0,
                           allow_small_or_imprecise_dtypes=True)
            # touch the ACT Square table so its ~2.7us load overlaps the
            # first feats DMAs instead of stalling the first ACT square
            warm = constp.tile([P, 1], fp32, tag="warm")
            nc.gpsimd.memset(warm[:], 0.0)
            nc.scalar.activation(warm[:], warm[:], Act.Square)
            # spin the PE through ~3.4us of dummy matmuls while the head
            # DMAs run: the HAM clock gate unthrottles (1.2 -> 2.4 GHz)
            # before the real stream starts instead of ~3.4us into it
            wmm = constp.tile([P, D], fp16, tag="wmm")
            nc.gpsimd.memset(wmm[:], 0.25)
            ps_warm = psp.tile([P, D], fp32, tag="warmmm", name="warmmm")
            for k in range(7):
                nc.tensor.matmul(out=ps_warm[:], lhsT=wmm[:, :P], rhs=wmm[:],
                                 start=k == 0, stop=k == 6)

            # per-group accumulators; a PSUM accumulation group owns its
            # whole bank, so each gets one
            ps_sums = [psp.tile([P, D], fp32, tag=f"sums{c}", name=f"sums{c}")
                       for c in range(2)]
            ps_ssq = [psp.tile([P, D], fp32, tag=f"ssq{c}",
                               name=f"ssq{c}") for c in range(2)]
            out_t = postp.tile([P, 4], fp32, tag="out_t")

            def readout(c):
                # norm2[g] and sumsq[g] for chunk c; chunk 0's runs
                # mid-loop, overlapped with chunk 1's compute, and each
                # chunk's stats go out on the (idle) scalar DMA ring
                scr_n = postp.tile([P, D], fp32, tag=f"scr_n{c}")
                nc.scalar.activation(scr_n[:], ps_sums[c][:], Act.Square,
                                     accum_out=out_t[:, 2 * c:2 * c + 1])
                nc.vector.tensor_reduce(
                    out=out_t[:, 2 * c + 1:2 * c + 2], in_=ps_ssq[c][:],
                    axis=mybir.AxisListType.X, op=Alu.add)
                nc.scalar.dma_start(out=stats[:, 2 * c:2 * c + 2],
                                    in_=out_t[:, 2 * c:2 * c + 2])

            # slab plan: tiny first slabs to start compute ASAP, tiny last
            # slab to shorten the post-loop tail
            if T >= 6:
                plan = [1, 3]
                rest = T - 4
                while rest > CH + 2:
                    plan.append(CH)
                    rest -= CH
                plan.extend([rest - 2, 2] if rest > 2 else [rest])
            else:
                plan = [1] * T

            t = 0
            for si, L in enumerate(plan):
                fx = fxp.tile([P, CH, D], fp16, tag="fx")
                nc.sync.dma_start(out=fx[:, :L, :],
                                  in_=feats_t[:, t:t + L, :])
                # fold Y on alternate slabs: trades DVE add-time for PE
                # stream-time so both sit just under the DMA floor; slabs
                # holding an accumulation-group start stay unfolded so
                # every PSUM column gets written each chunk
                fold = (si % 2 == 1 or si == len(plan) - 1) and not any(
                    t <= b < t + L for b in (0, T0 - 1, T0))
                ohs = []
                for j in range(L):
                    ti = t + j
                    # one-hot of this tile's local labels (exact in fp16)
                    oh = ohp.tile([P, P], fp16, tag="oh")
                    ohs.append(oh)
                    nc.vector.tensor_scalar(
                        out=oh[:], in0=iota[:], scalar1=labs[:, ti:ti + 1],
                        scalar2=None, op0=Alu.is_equal,
                    )
                    c = 0 if ti < T0 else 1
                    nc.tensor.matmul(out=ps_sums[c][:], lhsT=oh[:],
                                     rhs=fx[:, j, :],
                                     start=ti == 0 or ti == T0,
                                     stop=ti == T0 - 1 or ti == T - 1)
                # Y = X*X for the whole slab, column-split across engines
                yy = yyp.tile([P, CH, D], fp16, tag="yy")
                nc.vector.tensor_tensor(
                    out=yy[:, :L, :Y_WV], in0=fx[:, :L, :Y_WV],
                    in1=fx[:, :L, :Y_WV], op=Alu.mult)
                nc.scalar.activation(yy[:, :L, Y_WV:], fx[:, :L, Y_WV:],
                                     Act.Square)
                if fold:
                    yya = yyap.tile([P, CH, D // 2], fp16, tag="yya")
                    nc.vector.tensor_tensor(
                        out=yya[:, :L, :], in0=yy[:, :L, :D // 2],
                        in1=yy[:, :L, D // 2:], op=Alu.add)
                for j in range(L):
                    ti = t + j
                    c = 0 if ti < T0 else 1
                    if fold:
                        rhs, out = yya[:, j, :], ps_ssq[c][:, :D // 2]
                    else:
                        rhs, out = yy[:, j, :], ps_ssq[c][:]
                    nc.tensor.matmul(out=out, lhsT=ohs[j][:], rhs=rhs,
                                     start=ti == 0 or ti == T0,
                                     stop=ti == T0 - 1 or ti == T - 1,
                                     skip_group_check=True)
                if t < T0 <= t + L:  # chunk 0 just finished accumulating
                    readout(0)
                t += L

            readout(1)

    nc.compile()
    return nc


def _shard(feats, labels, demog):
    """Partition rows by (demog, label-half) across cores, then bucket by
    group-chunk (local label < 128 vs >= 128) within each core.

    Any row order within a bucket works: the device one-hot (iota vs
    label mod 128) routes each row to its group slot. Also returns the
    exact per-(core, chunk, slot) counts -- sharding metadata the host
    keeps for the finale.
    """
    half = (labels >= NL).astype(np.int32)
    shard_id = demog * 2 + half
    local = labels % NL          # 0..255 within the core
    chunk = local // P           # 0 or 1
    slot = (local % P).astype(np.float32)

    cnt = np.zeros((N_CORES, 2, P), np.int64)
    np.add.at(cnt, (shard_id, chunk, local % P), 1)

    feats16 = feats.astype(np.float16)
    rows0, rows1 = [], []
    for s in range(N_CORES):
        m = shard_id == s
        rows0.append(np.flatnonzero(m & (chunk == 0)))
        rows1.append(np.flatnonzero(m & (chunk == 1)))
    T0 = max(1, max(-(-len(r) // P) for r in rows0))
    T1 = max(1, max(-(-len(r) // P) for r in rows1))
    T = T0 + T1

    in_maps = []
    for s in range(N_CORES):
        f = np.zeros((T * P, D), np.float16)
        lab = np.full(T * P, PAD_LAB, np.float32)
        r0, r1 = rows0[s], rows1[s]
        f[:len(r0)] = feats16[r0]
        lab[:len(r0)] = slot[r0]
        f[T0 * P:T0 * P + len(r1)] = feats16[r1]
        lab[T0 * P:T0 * P + len(r1)] = slot[r1]
        # tile t, partition p <-> row t*128+p; device reads [P, T, D]
        ft = np.ascontiguousarray(f.reshape(T, P, D).transpose(1, 0, 2))
        lt = np.ascontiguousarray(lab.reshape(T, P).T)
        in_maps.append({"feats_t": ft, "labels_t": lt})
    return T0, T1, in_maps, cnt


def _combine(stats_per_core, cnt):
    """Host finale over the 2048 groups (the 'gather/unshard' step)."""
    num = np.zeros(ND, np.float64)
    den = np.zeros(ND, np.float64)
    for s, st in enumerate(stats_per_core):
        st = st.astype(np.float64)
        d = s // 2
        for c in range(2):
            norm2 = st[:, 2 * c]
            sumsq = st[:, 2 * c + 1]
            cg = cnt[s, c].astype(np.float64)
            safe = np.maximum(cg, 1.0)
            grp = (sumsq - norm2 / safe) / safe
            pres = cg > 0
            num[d] += grp[pres].sum()
            den[d] += pres.sum()
    intra = num / np.maximum(den, 1.0)
    return np.float32(np.mean(np.abs(intra - intra.mean())))


def kernel(feats, labels, demog_labels, _results_out=None):
    feats = np.ascontiguousarray(np.asarray(feats), dtype=np.float32)
    labels = np.asarray(labels).astype(np.int32)
    demog = np.asarray(demog_labels).astype(np.int32)
    assert feats.ndim == 2 and feats.shape[1] == D

    T0, T1, in_maps, cnt = _shard(feats, labels, demog)
    key = (T0, T1)
    nc = _cache.get(key)
    if nc is None:
        nc = _cache.setdefault(key, _build(T0, T1))
    res = None
    last_exc = None
    for attempt in range(3):
        try:
            res = bass_utils.run_bass_kernel_spmd(
                nc, in_maps, core_ids=list(range(N_CORES)))
            break
        except Exception as e:  # transient axon worker hangups
            last_exc = e
            import time
            time.sleep(10)
    if res is None:
        raise last_exc
    if _results_out is not None:
        _results_out.append(res)
    return _combine([res.results[s]["stats"] for s in range(N_CORES)], cnt)


# revision 23
# speedup vs baseline: 1.0586x; 1.0586x over previous
"""Trainium2 Bass kernel for nn_DebiasIntraDist (segment_reduce).

Full-input contract: kernel(**inputs) takes the complete (unsharded) inputs
and returns the full scalar loss. The N=65536 samples are sharded across the
8 NeuronCores by (demog, label-half): core 2d+h gets the rows with
demog == d and label-half h (a partition of the N axis). Every core then
owns a disjoint set of 256 (demog, label) groups, so no cross-core
reduction of group accumulators is needed at all.

Design (v10):
  * feats are quantized to fp16 on the host -> HBM traffic halves (the
    DMA floor is ~24 us/core). All on-device arithmetic on the quantized
    data is fp32-accumulated, so the only meaningful error is the fp16
    input rounding itself (~5e-4 relative on the loss).
  * within each core, rows are bucketed by group-chunk (local group id
    <128 vs >=128) so every 128-row tile touches a single 128-wide
    one-hot chunk -> each feature element streams through the PE exactly
    twice (sums and squares), not 4x as in the v1 baseline.
  * the one-hot matrices ride in from the host as uint8 and are cast to
    fp16 by the DMA engines (SWDGE cast path) -- zero compute-engine
    cost, vs ~12 us of 1x-mode is_equal on the Vector engine.
  * per-group sumsq uses a second PE matmul over Y = X*X. Y production
    is column-split between the Vector and Scalar engines; Y is folded
    in half (y[:256]+y[256:]) on most slabs so the sumsq matmul streams
    256 columns instead of 512, keeping the PE under the DMA floor.
  * group counts are sharding metadata; the host knows them exactly
    (bincount), so the device only accumulates sums[g,:] and sumsq[g].
  * no collective: each core DMAs out its 256 groups' statistics and the
    host does the final 2048-group scalar reduction (the v1 AllGather
    mesh cost ~35 us of serial tail).

Math per group: sums[g, :] and sumsqvec[g, :] (one-hot matmuls), then
  sum_{i in g} ||x_i - mu_g||^2 = sum_d sumsqvec[g,d] - ||sums[g]||^2/cnt[g].
"""

import numpy as np

try:
    import concourse.bacc as bacc
except ImportError:  # fresh environment without PYTHONPATH set up
    import sys
    for p in ("/root/.axon_site/_ro/trn_rl_repo", "/opt/trn_rl_repo",
              "/root/.axon_site/_ro/pypackages"):
        if p not in sys.path:
            sys.path.append(p)
    import concourse.bacc as bacc
import concourse.mybir as mybir
import concourse.tile as tile
import concourse.bass_utils as bass_utils

N_CORES = 8
P = 128
D = 512          # feature dim
NL = 256         # labels per core after (demog, label-half) sharding
ND = 4           # demog values
CH = 6           # sample-tiles per feats DMA (768 KiB)
Y_WV = 250       # columns of Y squared on VectorE (rest: ScalarE)

_cache: dict[tuple, object] = {}


def _build(T0: int, T1: int):
    """Compile the SPMD kernel: T0 tiles of chunk 0 then T1 of chunk 1."""
    T = T0 + T1
    fp32 = mybir.dt.float32
    fp16 = mybir.dt.float16
    u8 = mybir.dt.uint8
    Alu = mybir.AluOpType
    Act = mybir.ActivationFunctionType

    nc = bacc.Bacc("TRN2", target_bir_lowering=False, debug=False,
                   enable_asserts=True, num_devices=N_CORES)

    feats_t = nc.dram_tensor("feats_t", [P, T, D], fp16,
                             kind="ExternalInput").ap()
    oh_u8 = nc.dram_tensor("oh_u8", [P, T, P], u8, kind="ExternalInput").ap()
    stats = nc.dram_tensor("stats", [P, 4], fp32, kind="ExternalOutput").ap()

    with tile.TileContext(nc) as tc:
        with (
            tc.tile_pool(name="const", bufs=1) as constp,
            tc.tile_pool(name="fx", bufs=12) as fxp,
            tc.tile_pool(name="yy", bufs=4) as yyp,
            tc.tile_pool(name="yya", bufs=4) as yyap,
            tc.tile_pool(name="post", bufs=1) as postp,
            tc.tile_pool(name="ps", bufs=1, space="PSUM") as psp,
        ):
            # the whole one-hot table lives in SBUF (T*256 B/partition),
            # cast uint8 -> fp16 by the DMA engines; chunked so early
            # tiles are ready as soon as the engines come up
            oh_all = constp.tile([P, T, P], fp16, tag="oh_all")
            bounds = [0, 1, 8, 24, 44, T]
            for a, b in zip(bounds, bounds[1:]):
                if a < min(b, T):
                    nc.gpsimd.dma_start(out=oh_all[:, a:min(b, T), :],
                                        in_=oh_u8[:, a:min(b, T), :])
            # touch the ACT Square table so its ~2.7us load overlaps the
            # first feats DMAs instead of stalling the first ACT square
            warm = constp.tile([P, 1], fp32, tag="warm")
            nc.gpsimd.memset(warm[:], 0.0)
            nc.scalar.activation(warm[:], warm[:], Act.Square)

            # per-group accumulators; a PSUM accumulation group owns its
            # whole bank, so each gets one
            ps_sums = [psp.tile([P, D], fp32, tag=f"sums{c}", name=f"sums{c}")
                       for c in range(2)]
            ps_ssq = [psp.tile([P, D], fp32, tag=f"ssq{c}",
                               name=f"ssq{c}") for c in range(2)]
            out_t = postp.tile([P, 4], fp32, tag="out_t")

            def readout(c):
                # norm2[g] and sumsq[g] for chunk c; chunk 0's runs
                # mid-loop, overlapped with chunk 1's compute, and each
                # chunk's stats go out on the (idle) scalar DMA ring
                scr_n = postp.tile([P, D], fp32, tag=f"scr_n{c}")
                nc.scalar.activation(scr_n[:], ps_sums[c][:], Act.Square,
                                     accum_out=out_t[:, 2 * c:2 * c + 1])
                nc.vector.tensor_reduce(
                    out=out_t[:, 2 * c + 1:2 * c + 2], in_=ps_ssq[c][:],
                    axis=mybir.AxisListType.X, op=Alu.add)
                nc.scalar.dma_start(out=stats[:, 2 * c:2 * c + 2],
                                    in_=out_t[:, 2 * c:2 * c + 2])

            # slab plan: tiny first slabs to start compute ASAP, tiny last
            # slab to shorten the post-loop tail
            if T >= 6:
                plan = [1, 3]
                rest = T - 4
                while rest > CH + 2:
                    plan.append(CH)
                    rest -= CH
                plan.extend([rest - 2, 2] if rest > 2 else [rest])
            else:
                plan = [1] * T

            t = 0
            for si, L in enumerate(plan):
                fx = fxp.tile([P, CH, D], fp16, tag="fx")
                nc.sync.dma_start(out=fx[:, :L, :],
                                  in_=feats_t[:, t:t + L, :])
                # fold Y on most slabs (trades a cheap DVE add for PE
                # stream-time); the slab starting each chunk stays
                # unfolded so every PSUM column gets written each chunk
                fold = not any(t <= b < t + L for b in (0, T0))
                for j in range(L):
                    ti = t + j
                    c = 0 if ti < T0 else 1
                    nc.tensor.matmul(out=ps_sums[c][:],
                                     lhsT=oh_all[:, ti, :],
                                     rhs=fx[:, j, :],
                                     start=ti == 0 or ti == T0,
                                     stop=ti == T0 - 1 or ti == T - 1)
                # Y = X*X for the whole slab, column-split across engines
                yy = yyp.tile([P, CH, D], fp16, tag="yy")
                nc.vector.tensor_tensor(
                    out=yy[:, :L, :Y_WV], in0=fx[:, :L, :Y_WV],
                    in1=fx[:, :L, :Y_WV], op=Alu.mult)
                nc.scalar.activation(yy[:, :L, Y_WV:], fx[:, :L, Y_WV:],
                                     Act.Square)
                if fold:
                    yya = yyap.tile([P, CH, D // 2], fp16, tag="yya")
                    nc.vector.tensor_tensor(
                        out=yya[:, :L, :], in0=yy[:, :L, :D // 2],
                        in1=yy[:, :L, D // 2:], op=Alu.add)
                for j in range(L):
                    ti = t + j
                    c = 0 if ti < T0 else 1
                    if fold:
                        rhs, out = yya[:, j, :], ps_ssq[c][:, :D // 2]
                    else:
                        rhs, out = yy[:, j, :], ps_ssq[c][:]
                    nc.tensor.matmul(out=out, lhsT=oh_all[:, ti, :], rhs=rhs,
                                     start=ti == 0 or ti == T0,
                                     stop=ti == T0 - 1 or ti == T - 1,
                                     skip_group_check=True)
                if t < T0 <= t + L:  # chunk 0 just finished accumulating
                    readout(0)
                t += L

            readout(1)

    nc.compile()
    return nc


def _shard(feats, labels, demog):
    """Partition rows by (demog, label-half) across cores, then bucket by
    group-chunk (local label < 128 vs >= 128) within each core.

    Any row order within a bucket works: the host-built one-hot routes
    each row to its group slot. Also returns the exact per-(core, chunk,
    slot) counts -- sharding metadata the host keeps for the finale.
    """
    half = (labels >= NL).astype(np.int32)
    shard_id = demog * 2 + half
    local = labels % NL          # 0..255 within the core
    chunk = local // P           # 0 or 1
    slot = local % P

    cnt = np.zeros((N_CORES, 2, P), np.int64)
    np.add.at(cnt, (shard_id, chunk, slot), 1)

    feats16 = feats.astype(np.float16)
    rows0, rows1 = [], []
    for s in range(N_CORES):
        m = shard_id == s
        rows0.append(np.flatnonzero(m & (chunk == 0)))
        rows1.append(np.flatnonzero(m & (chunk == 1)))
    T0 = max(1, max(-(-len(r) // P) for r in rows0))
    T1 = max(1, max(-(-len(r) // P) for r in rows1))
    T = T0 + T1

    in_maps = []
    for s in range(N_CORES):
        f = np.zeros((T * P, D), np.float16)
        o = np.zeros((T * P, P), np.uint8)
        r0, r1 = rows0[s], rows1[s]
        f[:len(r0)] = feats16[r0]
        o[np.arange(len(r0)), slot[r0]] = 1
        f[T0 * P:T0 * P + len(r1)] = feats16[r1]
        o[T0 * P + np.arange(len(r1)), slot[r1]] = 1
        # tile t, partition p <-> row t*128+p; device reads [P, T, *]
        ft = np.ascontiguousarray(f.reshape(T, P, D).transpose(1, 0, 2))
        ot = np.ascontiguousarray(o.reshape(T, P, P).transpose(1, 0, 2))
        in_maps.append({"feats_t": ft, "oh_u8": ot})
    return T0, T1, in_maps, cnt


def _combine(stats_per_core, cnt):
    """Host finale over the 2048 groups (the 'gather/unshard' step)."""
    num = np.zeros(ND, np.float64)
    den = np.zeros(ND, np.float64)
    for s, st in enumerate(stats_per_core):
        st = st.astype(np.float64)
        d = s // 2
        for c in range(2):
            norm2 = st[:, 2 * c]
            sumsq = st[:, 2 * c + 1]
            cg = cnt[s, c].astype(np.float64)
            safe = np.maximum(cg, 1.0)
            grp = (sumsq - norm2 / safe) / safe
            pres = cg > 0
            num[d] += grp[pres].sum()
            den[d] += pres.sum()
    intra = num / np.maximum(den, 1.0)
    return np.float32(np.mean(np.abs(intra - intra.mean())))


def kernel(feats, labels, demog_labels, _results_out=None):
    feats = np.ascontiguousarray(np.asarray(feats), dtype=np.float32)
    labels = np.asarray(labels).astype(np.int32)
    demog = np.asarray(demog_labels).astype(np.int32)
    assert feats.ndim == 2 and feats.shape[1] == D

    T0, T1, in_maps, cnt = _shard(feats, labels, demog)
    key = (T0, T1)
    nc = _cache.get(key)
    if nc is None:
        nc = _cache.setdefault(key, _build(T0, T1))
    res = None
    last_exc = None
    for attempt in range(3):
        try:
            res = bass_utils.run_bass_kernel_spmd(
                nc, in_maps, core_ids=list(range(N_CORES)))
            break
        except Exception as e:  # transient axon worker hangups
            last_exc = e
            import time
            time.sleep(10)
    if res is None:
        raise last_exc
    if _results_out is not None:
        _results_out.append(res)
    return _combine([res.results[s]["stats"] for s in range(N_CORES)], cnt)


# revision 27
# speedup vs baseline: 1.1662x; 1.1016x over previous
"""Trainium2 Bass kernel for nn_DebiasIntraDist (segment_reduce).

Full-input contract: kernel(**inputs) takes the complete (unsharded) inputs
and returns the full scalar loss. The N=65536 samples are sharded across the
8 NeuronCores by (demog, label-half): core 2d+h gets the rows with
demog == d and label-half h (a partition of the N axis). Every core then
owns a disjoint set of 256 (demog, label) groups, so no cross-core
reduction of group accumulators is needed at all.

Design (v10):
  * feats are quantized to fp16 on the host -> HBM traffic halves (the
    DMA floor is ~24 us/core). All on-device arithmetic on the quantized
    data is fp32-accumulated, so the only meaningful error is the fp16
    input rounding itself (~5e-4 relative on the loss).
  * within each core, rows are bucketed by group-chunk (local group id
    <128 vs >=128) so every 128-row tile touches a single 128-wide
    one-hot chunk -> each feature element streams through the PE exactly
    twice (sums and squares), not 4x as in the v1 baseline.
  * the one-hot matrices ride in from the host as uint8 and are cast to
    fp16 by the DMA engines (SWDGE cast path) -- zero compute-engine
    cost, vs ~12 us of 1x-mode is_equal on the Vector engine.
  * per-group sumsq uses a second PE matmul over Y = X*X. Y production
    is column-split between the Vector and Scalar engines; Y is folded
    in half (y[:256]+y[256:]) on most slabs so the sumsq matmul streams
    256 columns instead of 512, keeping the PE under the DMA floor.
  * group counts are sharding metadata; the host knows them exactly
    (bincount), so the device only accumulates sums[g,:] and sumsq[g].
  * no collective: each core DMAs out its 256 groups' statistics and the
    host does the final 2048-group scalar reduction (the v1 AllGather
    mesh cost ~35 us of serial tail).

Math per group: sums[g, :] and sumsqvec[g, :] (one-hot matmuls), then
  sum_{i in g} ||x_i - mu_g||^2 = sum_d sumsqvec[g,d] - ||sums[g]||^2/cnt[g].
"""

import numpy as np

try:
    import concourse.bacc as bacc
except ImportError:  # fresh environment without PYTHONPATH set up
    import sys
    for p in ("/root/.axon_site/_ro/trn_rl_repo", "/opt/trn_rl_repo",
              "/root/.axon_site/_ro/pypackages"):
        if p not in sys.path:
            sys.path.append(p)
    import concourse.bacc as bacc
import concourse.mybir as mybir
import concourse.tile as tile
import concourse.bass_utils as bass_utils

N_CORES = 8
P = 128
D = 512          # feature dim
NL = 256         # labels per core after (demog, label-half) sharding
ND = 4           # demog values
CH = 6           # sample-tiles per feats DMA (768 KiB)
Y_WV = 176       # columns of Y squared on VectorE (rest: ScalarE)

_cache: dict[tuple, object] = {}


def _build(T0: int, T1: int):
    """Compile the SPMD kernel: T0 tiles of chunk 0 then T1 of chunk 1."""
    T = T0 + T1
    fp32 = mybir.dt.float32
    fp16 = mybir.dt.float16
    u8 = mybir.dt.uint8
    Alu = mybir.AluOpType
    Act = mybir.ActivationFunctionType

    nc = bacc.Bacc("TRN2", target_bir_lowering=False, debug=False,
                   enable_asserts=True, num_devices=N_CORES)

    feats_t = nc.dram_tensor("feats_t", [P, T, D], fp16,
                             kind="ExternalInput").ap()
    oh_u8 = nc.dram_tensor("oh_u8", [P, T, P], u8, kind="ExternalInput").ap()
    stats = nc.dram_tensor("stats", [P, 4], fp32, kind="ExternalOutput").ap()

    with tile.TileContext(nc) as tc:
        with (
            tc.tile_pool(name="const", bufs=1) as constp,
            tc.tile_pool(name="fx", bufs=12) as fxp,
            tc.tile_pool(name="ou", bufs=4) as oup,
            tc.tile_pool(name="oh", bufs=4) as ohp,
            tc.tile_pool(name="yy", bufs=4) as yyp,
            tc.tile_pool(name="yya", bufs=4) as yyap,
            tc.tile_pool(name="post", bufs=1) as postp,
            tc.tile_pool(name="ps", bufs=1, space="PSUM") as psp,
        ):
            # touch the ACT Square table so its ~2.7us load overlaps the
            # first feats DMAs instead of stalling the first ACT square
            warm = constp.tile([P, 1], fp32, tag="warm")
            nc.gpsimd.memset(warm[:], 0.0)
            nc.scalar.activation(warm[:], warm[:], Act.Square)
            # spin the PE through dummy matmuls while the head DMAs run:
            # the HAM clock gate unthrottles (1.2 -> 2.4 GHz) before the
            # real stream starts instead of ~3.4us into it
            wmm = constp.tile([P, D], fp16, tag="wmm")
            nc.gpsimd.memset(wmm[:], 0.25)
            ps_warm = psp.tile([P, D], fp32, tag="warmmm", name="warmmm")
            for k in range(4):
                nc.tensor.matmul(out=ps_warm[:], lhsT=wmm[:, :P], rhs=wmm[:],
                                 start=k == 0, stop=k == 3)

            # per-group accumulators; a PSUM accumulation group owns its
            # whole bank, so each gets one
            ps_sums = [psp.tile([P, D], fp32, tag=f"sums{c}", name=f"sums{c}")
                       for c in range(2)]
            ps_ssq = [psp.tile([P, D], fp32, tag=f"ssq{c}",
                               name=f"ssq{c}") for c in range(2)]
            out_t = postp.tile([P, 4], fp32, tag="out_t")

            def readout(c):
                # norm2[g] and sumsq[g] for chunk c; chunk 0's runs
                # mid-loop, overlapped with chunk 1's compute, and each
                # chunk's stats go out on the (idle) scalar DMA ring
                scr_n = postp.tile([P, D], fp32, tag=f"scr_n{c}")
                nc.scalar.activation(scr_n[:], ps_sums[c][:], Act.Square,
                                     accum_out=out_t[:, 2 * c:2 * c + 1])
                nc.vector.tensor_reduce(
                    out=out_t[:, 2 * c + 1:2 * c + 2], in_=ps_ssq[c][:],
                    axis=mybir.AxisListType.X, op=Alu.add)
                nc.scalar.dma_start(out=stats[:, 2 * c:2 * c + 2],
                                    in_=out_t[:, 2 * c:2 * c + 2])

            # slab plan: tiny first slabs to start compute ASAP, tiny last
            # slab to shorten the post-loop tail
            if T >= 6:
                plan = [1, 3]
                rest = T - 4
                while rest > CH + 2:
                    plan.append(CH)
                    rest -= CH
                plan.extend([rest - 2, 2] if rest > 2 else [rest])
            else:
                plan = [1] * T

            t = 0
            for si, L in enumerate(plan):
                # the slab's one-hots ride the sync ring just ahead of its
                # features, then a single 2x-mode copy casts them to fp16
                ou = oup.tile([P, CH, P], u8, tag="ou")
                nc.sync.dma_start(out=ou[:, :L, :], in_=oh_u8[:, t:t + L, :])
                fx = fxp.tile([P, CH, D], fp16, tag="fx")
                nc.sync.dma_start(out=fx[:, :L, :],
                                  in_=feats_t[:, t:t + L, :])
                oh = ohp.tile([P, CH, P], fp16, tag="oh")
                nc.vector.tensor_copy(out=oh[:, :L, :], in_=ou[:, :L, :])
                # fold Y on most slabs (trades a cheap DVE add for PE
                # stream-time); the slab starting each chunk stays
                # unfolded so every PSUM column gets written each chunk
                fold = not any(t <= b < t + L for b in (0, T0))
                for j in range(L):
                    ti = t + j
                    c = 0 if ti < T0 else 1
                    nc.tensor.matmul(out=ps_sums[c][:],
                                     lhsT=oh[:, j, :],
                                     rhs=fx[:, j, :],
                                     start=ti == 0 or ti == T0,
                                     stop=ti == T0 - 1 or ti == T - 1)
                # Y = X*X for the whole slab, column-split across engines
                yy = yyp.tile([P, CH, D], fp16, tag="yy")
                nc.vector.tensor_tensor(
                    out=yy[:, :L, :Y_WV], in0=fx[:, :L, :Y_WV],
                    in1=fx[:, :L, :Y_WV], op=Alu.mult)
                nc.scalar.activation(yy[:, :L, Y_WV:], fx[:, :L, Y_WV:],
                                     Act.Square)
                if fold:
                    yya = yyap.tile([P, CH, D // 2], fp16, tag="yya")
                    nc.vector.tensor_tensor(
                        out=yya[:, :L, :], in0=yy[:, :L, :D // 2],
                        in1=yy[:, :L, D // 2:], op=Alu.add)
                for j in range(L):
                    ti = t + j
                    c = 0 if ti < T0 else 1
                    if fold:
                        rhs, out = yya[:, j, :], ps_ssq[c][:, :D // 2]
                    else:
                        rhs, out = yy[:, j, :], ps_ssq[c][:]
                    nc.tensor.matmul(out=out, lhsT=oh[:, j, :], rhs=rhs,
                                     start=ti == 0 or ti == T0,
                                     stop=ti == T0 - 1 or ti == T - 1,
                                     skip_group_check=True)
                if t < T0 <= t + L:  # chunk 0 just finished accumulating
                    readout(0)
                t += L

            readout(1)

    nc.compile()
    return nc


def _shard(feats, labels, demog):
    """Partition rows by (demog, label-half) across cores, then bucket by
    group-chunk (local label < 128 vs >= 128) within each core.

    Any row order within a bucket works: the host-built one-hot routes
    each row to its group slot. Also returns the exact per-(core, chunk,
    slot) counts -- sharding metadata the host keeps for the finale.
    """
    half = (labels >= NL).astype(np.int32)
    shard_id = demog * 2 + half
    local = labels % NL          # 0..255 within the core
    chunk = local // P           # 0 or 1
    slot = local % P

    cnt = np.zeros((N_CORES, 2, P), np.int64)
    np.add.at(cnt, (shard_id, chunk, slot), 1)

    feats16 = feats.astype(np.float16)
    rows0, rows1 = [], []
    for s in range(N_CORES):
        m = shard_id == s
        rows0.append(np.flatnonzero(m & (chunk == 0)))
        rows1.append(np.flatnonzero(m & (chunk == 1)))
    T0 = max(1, max(-(-len(r) // P) for r in rows0))
    T1 = max(1, max(-(-len(r) // P) for r in rows1))
    T = T0 + T1

    in_maps = []
    for s in range(N_CORES):
        f = np.zeros((T * P, D), np.float16)
        o = np.zeros((T * P, P), np.uint8)
        r0, r1 = rows0[s], rows1[s]
        f[:len(r0)] = feats16[r0]
        o[np.arange(len(r0)), slot[r0]] = 1
        f[T0 * P:T0 * P + len(r1)] = feats16[r1]
        o[T0 * P + np.arange(len(r1)), slot[r1]] = 1
        # tile t, partition p <-> row t*128+p; device reads [P, T, *]
        ft = np.ascontiguousarray(f.reshape(T, P, D).transpose(1, 0, 2))
        ot = np.ascontiguousarray(o.reshape(T, P, P).transpose(1, 0, 2))
        in_maps.append({"feats_t": ft, "oh_u8": ot})
    return T0, T1, in_maps, cnt


def _combine(stats_per_core, cnt):
    """Host finale over the 2048 groups (the 'gather/unshard' step)."""
    num = np.zeros(ND, np.float64)
    den = np.zeros(ND, np.float64)
    for s, st in enumerate(stats_per_core):
        st = st.astype(np.float64)
        d = s // 2
        for c in range(2):
            norm2 = st[:, 2 * c]
            sumsq = st[:, 2 * c + 1]
            cg = cnt[s, c].astype(np.float64)
            safe = np.maximum(cg, 1.0)
            grp = (sumsq - norm2 / safe) / safe
            pres = cg > 0
            num[d] += grp[pres].sum()
            den[d] += pres.sum()
    intra = num / np.maximum(den, 1.0)
    return np.float32(np.mean(np.abs(intra - intra.mean())))


def kernel(feats, labels, demog_labels, _results_out=None):
    feats = np.ascontiguousarray(np.asarray(feats), dtype=np.float32)
    labels = np.asarray(labels).astype(np.int32)
    demog = np.asarray(demog_labels).astype(np.int32)
    assert feats.ndim == 2 and feats.shape[1] == D

    T0, T1, in_maps, cnt = _shard(feats, labels, demog)
    key = (T0, T1)
    nc = _cache.get(key)
    if nc is None:
        nc = _cache.setdefault(key, _build(T0, T1))
    res = None
    last_exc = None
    for attempt in range(3):
        try:
            res = bass_utils.run_bass_kernel_spmd(
                nc, in_maps, core_ids=list(range(N_CORES)))
            break
        except Exception as e:  # transient axon worker hangups
            last_exc = e
            import time
            time.sleep(10)
    if res is None:
        raise last_exc
    if _results_out is not None:
        _results_out.append(res)
    return _combine([res.results[s]["stats"] for s in range(N_CORES)], cnt)
